# revision 4
# baseline (speedup 1.0000x reference)
"""DEDICOM decoder edge scoring on 8 TRN2 NeuronCores.

scores[e] = (z[src_e] * d) @ R @ (z[dst_e] * d)  for 1M edges.

Strategy (data-parallel over edges, z/R/D replicated):
  - device precomputes M = d (x) d * R, then the table Y = z @ M (HBM);
  - per 2048-edge chunk: dma_gather Y[src] and z[dst] rows (512 B each)
    striped over 4 SWDGE queues, then a fused DVE multiply+reduce gives
    the per-edge dot products.
  - dma_gather indices are int16, so tables are addressed in two halves
    (rows < 32000 and >= 32000); the host buckets each core's edges by
    (src half, dst half) and un-permutes the scores afterwards.
"""
import numpy as np
import concourse.bacc as bacc
import concourse.mybir as mybir
from concourse.tile import TileContext
from concourse.bass_utils import run_bass_kernel_spmd
N_CORES = 8
N_NODES = 50000
D = 128
HALF = 32000          # int16-safe table split point
CHUNK = 2048          # edges per dma_gather call
NQ = 1                # single SWDGE queue: Tile's DMA-sem lanes lock per queue


def _build_program(nchunks_per_bucket):
    total_chunks = sum(nchunks_per_bucket)
    ntot = total_chunks * CHUNK
    nc = bacc.Bacc("TRN2", num_devices=N_CORES, num_swdge_queues=NQ)
    z = nc.declare_dram_parameter("z", [N_NODES, D], mybir.dt.float32, isOutput=False)
    R = nc.declare_dram_parameter("R", [D, D], mybir.dt.float32, isOutput=False)
    dr = nc.declare_dram_parameter("dr", [1, D], mybir.dt.float32, isOutput=False)
    identity = nc.declare_dram_parameter("ident", [128, 128], mybir.dt.float32, isOutput=False)
    isrc = nc.declare_dram_parameter("isrc", [128, ntot // 16], mybir.dt.int16, isOutput=False)
    idst = nc.declare_dram_parameter("idst", [128, ntot // 16], mybir.dt.int16, isOutput=False)
    scores = nc.declare_dram_parameter("scores", [128, ntot // 128], mybir.dt.float32, isOutput=True)
    Y = nc.dram_tensor("Ytab", [N_NODES, D], mybir.dt.float32)

    with TileContext(nc) as tc:
        with (
            tc.tile_pool(name="const", bufs=1) as constp,
            tc.tile_pool(name="drps", bufs=1, space="PSUM") as drpsp,
            tc.tile_pool(name="ypsum", bufs=2, space="PSUM") as ypsum,
            tc.tile_pool(name="ywork", bufs=3) as ywork,
            tc.tile_pool(name="idxp", bufs=1) as idxp,
            tc.tile_pool(name="gat", bufs=4) as gatp,
            tc.tile_pool(name="dot", bufs=2) as dotp,
            tc.tile_pool(name="scorep", bufs=1) as scorep,
        ):
            # ---- constants: identity, R, d_r, M = (d (x) d) * R ----
            ident = constp.tile([128, 128], mybir.dt.float32)
            nc.sync.dma_start(out=ident[:], in_=identity[:])
            R_sb = constp.tile([128, D], mybir.dt.float32)
            nc.sync.dma_start(out=R_sb[:], in_=R[:])
            dr_sb = constp.tile([1, D], mybir.dt.float32)
            nc.sync.dma_start(out=dr_sb[:], in_=dr[:])
            DRps = drpsp.tile([128, 128], mybir.dt.float32)
            nc.tensor.matmul(out=DRps[:], lhsT=dr_sb[:], rhs=dr_sb[:], start=True, stop=True)
            M_sb = constp.tile([128, D], mybir.dt.float32)
            nc.vector.tensor_tensor(out=M_sb[:], in0=R_sb[:], in1=DRps[:], op=mybir.AluOpType.mult)

            # ---- Y = z @ M, built 128 rows at a time ----
            nrow_chunks = (N_NODES + 127) // 128
            for ci in range(nrow_chunks):
                r0 = ci * 128
                rows = min(128, N_NODES - r0)
                zt = ywork.tile([128, D], mybir.dt.float32, tag="zt")
                nc.sync.dma_start(out=zt[:rows, :], in_=z[r0:r0 + rows, :])
                zT_ps = ypsum.tile([128, 128], mybir.dt.float32, tag="zT")
                nc.tensor.transpose(out=zT_ps[:, :rows], in_=zt[:rows, :],
                                    identity=ident[:rows, :rows])
                zT_sb = ywork.tile([128, 128], mybir.dt.float32, tag="zTsb")
                nc.vector.tensor_copy(out=zT_sb[:, :rows], in_=zT_ps[:, :rows])
                yT_ps = ypsum.tile([128, 128], mybir.dt.float32, tag="yT")
                nc.tensor.matmul(out=yT_ps[:, :rows], lhsT=M_sb[:], rhs=zT_sb[:, :rows],
                                 start=True, stop=True)
                yT_sb = ywork.tile([128, 128], mybir.dt.float32, tag="yTsb")
                nc.vector.tensor_copy(out=yT_sb[:, :rows], in_=yT_ps[:, :rows])
                y_ps = ypsum.tile([128, 128], mybir.dt.float32, tag="yrm")
                nc.tensor.transpose(out=y_ps[:rows, :], in_=yT_sb[:, :rows],
                                    identity=ident[:])
                y_sb = ywork.tile([128, D], mybir.dt.float32, tag="ysb")
                nc.vector.tensor_copy(out=y_sb[:rows, :], in_=y_ps[:rows, :])
                nc.sync.dma_start(out=Y[r0:r0 + rows, :], in_=y_sb[:rows, :])

            # ---- main loop: gather + fused dot ----
            isrc_sb = idxp.tile([128, ntot // 16], mybir.dt.int16)
            nc.sync.dma_start(out=isrc_sb[:], in_=isrc[:])
            idst_sb = idxp.tile([128, ntot // 16], mybir.dt.int16)
            nc.sync.dma_start(out=idst_sb[:], in_=idst[:])
            score_sb = scorep.tile([128, ntot // 128], mybir.dt.float32)

            k = 0
            for b in range(4):
                src_t = Y[:, :] if b < 2 else Y[HALF:, :]
                dst_t = z[:, :] if b % 2 == 0 else z[HALF:, :]
                for _ in range(nchunks_per_bucket[b]):
                    c16 = k * (CHUNK // 16)
                    g1 = gatp.tile([128, CHUNK // 128, D], mybir.dt.float32, tag="g1")
                    nc.gpsimd.dma_gather(
                        g1[:], src_t, isrc_sb[:, c16:c16 + CHUNK // 16],
                        CHUNK, CHUNK, D, single_packet=False, queue_num=(2 * k) % NQ)
                    g2 = gatp.tile([128, CHUNK // 128, D], mybir.dt.float32, tag="g2")
                    nc.gpsimd.dma_gather(
                        g2[:], dst_t, idst_sb[:, c16:c16 + CHUNK // 16],
                        CHUNK, CHUNK, D, single_packet=False, queue_num=(2 * k + 1) % NQ)
                    prod = dotp.tile([128, CHUNK // 128, D], mybir.dt.float32, tag="prod")
                    nc.vector.tensor_tensor(
                        out=prod[:], in0=g1[:], in1=g2[:], op=mybir.AluOpType.mult)
                    nc.vector.tensor_reduce(
                        out=score_sb[:, k * 16:(k + 1) * 16], in_=prod[:],
                        axis=mybir.AxisListType.X, op=mybir.AluOpType.add)
                    k += 1
            nc.sync.dma_start(out=scores[:], in_=score_sb[:])
    nc.compile()
    return nc


def _prepare(inputs):
    z = np.ascontiguousarray(np.asarray(inputs["z"], dtype=np.float32))
    R = np.ascontiguousarray(np.asarray(inputs["R"], dtype=np.float32))
    Dm = np.asarray(inputs["D"], dtype=np.float32)
    edge_index = np.asarray(inputs["edge_index"])
    rel = int(np.asarray(inputs["relation_idx"]))
    dr = np.ascontiguousarray(Dm[rel:rel + 1, :])

    B = edge_index.shape[1]
    assert B % N_CORES == 0
    per = B // N_CORES
    src_all = edge_index[0].astype(np.int64)
    dst_all = edge_index[1].astype(np.int64)

    cores = []
    counts = np.zeros((N_CORES, 4), np.int64)
    for c in range(N_CORES):
        s = src_all[c * per:(c + 1) * per]
        d = dst_all[c * per:(c + 1) * per]
        bkey = (s >= HALF).astype(np.int64) * 2 + (d >= HALF).astype(np.int64)
        order = np.argsort(bkey, kind="stable")
        cores.append((s[order], d[order], order))
        counts[c] = np.bincount(bkey, minlength=4)
    nch = [int(np.ceil(counts[:, b].max() / CHUNK)) for b in range(4)]
    ntot = sum(nch) * CHUNK

    def wrap(a):
        w = np.ascontiguousarray(a.reshape(-1, 16).T.astype(np.int16))
        return np.tile(w, (8, 1))

    in_maps = []
    for c in range(N_CORES):
        ssorted, dsorted, _ = cores[c]
        sarr = np.zeros(ntot, np.int64)
        darr = np.zeros(ntot, np.int64)
        off_in = 0
        off_out = 0
        for b in range(4):
            n = int(counts[c, b])
            sarr[off_out:off_out + n] = ssorted[off_in:off_in + n] - (HALF if b >= 2 else 0)
            darr[off_out:off_out + n] = dsorted[off_in:off_in + n] - (HALF if b % 2 else 0)
            off_in += n
            off_out += nch[b] * CHUNK
        in_maps.append({"z": z, "R": R, "dr": dr,
                        "ident": np.eye(128, dtype=np.float32),
                        "isrc": wrap(sarr), "idst": wrap(darr)})
    return in_maps, cores, counts, nch, ntot, per, B


def _collect(res, cores, counts, nch, ntot, per, B):
    out = np.empty(B, np.float32)
    nchunks = ntot // CHUNK
    for c in range(N_CORES):
        sc = np.asarray(res.results[c]["scores"])  # [128, ntot//128]
        padded = sc.reshape(128, nchunks, 16).transpose(1, 2, 0).reshape(-1)
        _, _, order = cores[c]
        vals = np.empty(per, np.float32)
        off_in = 0
        off_out = 0
        for b in range(4):
            n = int(counts[c, b])
            vals[off_in:off_in + n] = padded[off_out:off_out + n]
            off_in += n
            off_out += nch[b] * CHUNK
        outslice = np.empty(per, np.float32)
        outslice[order] = vals
        out[c * per:(c + 1) * per] = outslice
    return out


def kernel_with_time(inputs, trace=False):
    in_maps, cores, counts, nch, ntot, per, B = _prepare(inputs)
    nc = _build_program(nch)
    res = run_bass_kernel_spmd(nc, in_maps, list(range(N_CORES)), trace=trace)
    out = _collect(res, cores, counts, nch, ntot, per, B)
    return out, res.exec_time_ns


def kernel(**inputs):
    out, _ = kernel_with_time(inputs, trace=False)
    return out
